# revision 28
# baseline (speedup 1.0000x reference)
"""Trainium2 Bass kernel for nn_CausalAttentionPooling.

Math: scores[b,i,j] = x[b,i].q are constant along the softmax axis j, so
softmax over the causal mask yields uniform weights 1/(i+1) on j <= i.
The module is exactly a causal cumulative mean:
    out[b,i,:] = cumsum(x, axis=1)[b,i,:] / (i+1)
(q does not affect the output.)

Sharding: 8 shards = (batch b in 0..3) x (D-half dh in 0..1); each core gets
x[b, :, dh*128:(dh+1)*128] transposed to [128(D), 4096(L)].  Per core:
  - DVE tensor_tensor_scan along the free dim -> exact fp32 cumsum
  - DVE tensor_tensor multiply by a replicated 1/(i+1) row (host-shipped)
  - DMA out [128(D), 4096(L)]; host transposes slices back
No cross-core communication; DMA count kept minimal (issue cost ~600ns each).
"""

import numpy as np

B, L, D = 4, 4096, 256
NCORES = 8
P = 128            # partitions / D-shard width

_cache = {}


def _split_waits_bir(bir_bytes):
    """This container's walrus build rejects instructions carrying more than
    one (or for some opcodes, two) sync waits.  Hoist multi-wait sync_info
    onto standalone same-engine EventSemaphore instructions inserted
    immediately before the instruction; program order on the engine's stream
    preserves semantics."""
    import orjson

    d = orjson.loads(bir_bytes)
    n = 0
    for fn in d["functions"]:
        for bb in fn["blocks"]:
            out = []
            for inst in bb["instructions"]:
                si = inst.get("sync_info")
                waits = (si or {}).get("on_wait") or []
                if len(waits) > 1:
                    for w in waits:
                        out.append(
                            {
                                "debug": inst.get("debug"),
                                "engine": inst["engine"],
                                "ins": [],
                                "name": f"I-waitfix-{n}",
                                "opcode": "EventSemaphore",
                                "outs": [],
                                "sync_info": {"on_wait": [w], "on_update": []},
                            }
                        )
                        n += 1
                    si["on_wait"] = []
                out.append(inst)
            bb["instructions"] = out
    return orjson.dumps(d)


def _install_bir_patch():
    if _cache.get("patched"):
        return
    import concourse.bass as bass

    orig = bass.Bass.to_json_bytes

    def patched(self):
        return _split_waits_bir(orig(self))

    bass.Bass.to_json_bytes = patched
    _cache["patched"] = True


def _build_nc():
    import concourse.bass as bass
    import concourse.tile as tile
    from concourse import mybir

    _install_bir_patch()

    f32 = mybir.dt.float32
    add = mybir.AluOpType.add
    byp = mybir.AluOpType.bypass
    mult = mybir.AluOpType.mult

    nc = bass.Bass()
    xT = nc.declare_dram_parameter("xT", [P, L], f32, isOutput=False)
    rrow = nc.declare_dram_parameter("rrow", [1, L], f32, isOutput=False)
    # replicated tail chunks of 1/(i+1): PE production can't keep up with the
    # DVE at the tail, so the last PE_SPLIT..NB-1 chunks come from HBM instead
    rrt = nc.declare_dram_parameter("rrt", [P, L - 7 * 512], f32, isOutput=False)
    out = nc.declare_dram_parameter("out", [P, L], f32, isOutput=True)

    PB = 512  # psum bank free size (fp32) == scan/mult block
    NB = L // PB
    PE_SPLIT = 8  # chunks [0, PE_SPLIT) from PE broadcast, rest from HBM

    with tile.TileContext(nc) as tc:
        with (
            tc.tile_pool(name="sb", bufs=1) as sb,
            tc.tile_pool(name="ps", bufs=1, space="PSUM") as ps,
        ):
            xt = sb.tile([P, L], f32, tag="xt")
            cum = sb.tile([P, L], f32, tag="cum")
            ot = sb.tile([P, L], f32, tag="ot")
            rrow_sb = sb.tile([1, PE_SPLIT * PB], f32, tag="rrow")
            rrt_sb = (
                sb.tile([P, L - PE_SPLIT * PB], f32, tag="rrt")
                if PE_SPLIT < NB
                else None
            )
            ones = sb.tile([1, P], f32, tag="ones")

            # x block 0 split small so the first scan starts early; later
            # spans are wide so DMA descriptors are 4-14KB (not 2KB) and the
            # aggregate read rate stays near the HBM roofline.  Later spans
            # are throttled behind early scans (DMA engines round-robin all
            # in-flight transfers, so eagerly issuing everything starves the
            # spans the DVE needs first).
            # tiny rrow first so the PE broadcast isn't stuck behind 2MB of x
            nc.sync.dma_start(rrow_sb[:], rrow[:, : PE_SPLIT * PB])
            xsplits = [0, 128, 512, 1024, 1536, 2560, 3584, L]
            xdmas = []
            for a, b in zip(xsplits[:-1], xsplits[1:]):
                xdmas.append(nc.sync.dma_start(xt[:, a:b], xT[:, a:b]))
            # replicated tail scale chunks issued last (needed last)
            if PE_SPLIT < NB:
                nc.sync.dma_start(rrt_sb[:], rrt[:])
            nc.vector.memset(ones[:], 1.0)

            # replicate 1/(i+1) across partitions on the idle PE:
            # outer product ones[1,128].T @ rrow[1,512] -> psum [128,512]
            rr_ps = []
            for j in range(PE_SPLIT):
                pt = ps.tile([P, PB], f32, tag=f"rr{j}")
                nc.tensor.matmul(
                    pt[:],
                    ones[:],
                    rrow_sb[:, j * PB : (j + 1) * PB],
                    start=True,
                    stop=True,
                )
                rr_ps.append(pt)

            def rr_of(k):
                if k < PE_SPLIT:
                    return rr_ps[k][:]
                return rrt_sb[:, (k - PE_SPLIT) * PB : (k - PE_SPLIT + 1) * PB]

            scan_insts = []

            def scan(a, b):
                init = 0.0 if a == 0 else cum[:, a - 1 : a]
                scan_insts.append(
                    nc.vector.tensor_tensor_scan(
                        cum[:, a:b], xt[:, a:b], xt[:, a:b], init, op0=add, op1=byp
                    )
                )

            def mult_out(k, a=None, b=None):
                a = k * PB if a is None else a
                b = (k + 1) * PB if b is None else b
                ro = rr_of(k)
                ro = ro[:, a - k * PB : b - k * PB]
                nc.vector.tensor_tensor(ot[:, a:b], cum[:, a:b], ro, op=mult)
                # out DMA issued from the otherwise-idle Scalar engine so the
                # Sync sequencer's descriptor-gen doesn't serialize with input
                nc.scalar.dma_start(out[:, a:b], ot[:, a:b])

            # scan blocks follow the DMA spans; mults (512-wide, matching the
            # psum rr chunks) are emitted once their span's scan is done and
            # staggered so the DVE never stalls on the PE's first rr chunk
            scan(0, 128)
            scan(128, 512)
            scan(512, 1024)
            mult_out(0)
            scan(1024, 1536)
            mult_out(1)
            scan(1536, 2560)
            mult_out(2)
            mult_out(3)
            scan(2560, 3584)
            mult_out(4)
            mult_out(5)
            scan(3584, 3840)
            mult_out(6)
            scan(3840, L)
            mult_out(7, 3584, 3840)
            mult_out(7, 3840, L)
    return nc


def _get_nc():
    if "nc" not in _cache:
        _cache["nc"] = _build_nc()
    return _cache["nc"]


def _make_in_maps(x):
    idx = np.arange(1, L + 1, dtype=np.float64)
    rrow = (1.0 / idx).astype(np.float32).reshape(1, L)
    rrt = np.ascontiguousarray(np.broadcast_to(rrow[:, 7 * 512 :], (P, L - 7 * 512)))
    in_maps = []
    shards = []
    for c in range(NCORES):
        b, dh = c // 2, c % 2
        shards.append((b, dh))
        xT = np.ascontiguousarray(x[b, :, dh * P : (dh + 1) * P].T)
        in_maps.append({"xT": xT, "rrow": rrow, "rrt": rrt})
    return in_maps, shards


def kernel(x, q):
    from concourse.bass_utils import run_bass_kernel_spmd

    x = np.asarray(x)
    assert x.shape == (B, L, D) and x.dtype == np.float32

    nc = _get_nc()
    in_maps, shards = _make_in_maps(x)
    results = run_bass_kernel_spmd(nc, in_maps, list(range(NCORES))).results

    out = np.empty((B, L, D), dtype=np.float32)
    for c, (b, dh) in enumerate(shards):
        out[b, :, dh * P : (dh + 1) * P] = results[c]["out"].T
    return out
